# revision 14
# baseline (speedup 1.0000x reference)
"""Self-contained Trainium2 Bass kernel for a 2-layer GCN + FC + log_softmax.

Distribution: nodes sharded across 8 NeuronCores (12500 rows each); edges
partitioned by destination node so each core's scatter-add is local; the
per-layer "gather tables" g = D^-1/2 * H * W are exchanged with an on-chip
AllGather; small weights replicated.

Device algorithm per core:
  Stage A : g1 slice = (dinv*x) @ W1 (rows of this core), fp16 -> AllGather
  Agg     : per 125-node dst tile: PSUM += onehot(seg).T @ g1[src]
            (dma_gather of fp16 rows from 4 HBM banks on 4 SWDGE queues,
            one-hot built on VectorE from preloaded seg values, segment-sum
            as TensorE matmul).  Self-loops are NOT gathered: they are a
            diag(dinv) matmul against the core-local g rows.
  Flush B : h1 = relu(dinv*acc); g2 = (dinv*h1) @ W2 -> AllGather
  Flush C : h2 = relu(dinv*acc); logits = h2 @ Wfc; fused log_softmax.
"""
import hashlib
import math

import numpy as np
import ml_dtypes

import concourse.bass as bass
import concourse.mybir as mybir
import concourse.tile as tile
from concourse import bacc, bass_utils

FP16_NP = ml_dtypes.float16 if hasattr(ml_dtypes, "float16") else np.float16

# Problem contract (hardcoded; must match setup_inputs()).
N_NODES = 100000
N_EDGES = 1600000
D = 128
DOUT = 40

N_CORES = 8
TILE_N = 125            # dst nodes per PSUM tile
TB = 10                 # dst tiles per batch
BANK = 25000            # gather table bank rows (int16 index limit 32767)
HALF = 6250             # node rows per core per table half (AG split)
CHUNK = 128             # edges per matmul chunk
MAXC = 12               # chunks per dma_gather call
KM = 8                  # max chunks per one-hot build run
AHEAD = 2               # gather issue lookahead (batches)
PAR_OFF = 500.0         # seg offset for odd tiles (collision guard)
SENT = 1250.0           # seg sentinel (matches no iota value)
OHW = 126               # one-hot row width (125 cols + 1 pad col)

FP32 = mybir.dt.float32
FP16 = mybir.dt.float16
I16 = mybir.dt.int16

SLICE_N = N_NODES // N_CORES            # 12500
QROWS = SLICE_N // 4                    # local rows per AG quarter
N_TILES = SLICE_N // TILE_N             # 100
N_BATCH = N_TILES // TB                 # 10
N_BANKS = (N_NODES + BANK - 1) // BANK  # 4
TPQ = 25                                # dst tiles per bounce quarter


# ---------------------------------------------------------------------------
# Host preprocessing
# ---------------------------------------------------------------------------

def _preprocess(edge_index):
    """Sort edges by (core, batch, bank, tile); build the static chunk
    structure shared by both conv layers plus per-core idx/seg arrays."""
    ei = np.asarray(edge_index, np.int64)
    src, dst = ei[0], ei[1]
    deg = (np.bincount(dst, minlength=N_NODES) + 1.0).astype(np.float32)
    dinv = 1.0 / np.sqrt(deg)
    sqdeg = np.sqrt(deg)

    tile_id = dst // TILE_N                  # global tile 0..799
    core_id = tile_id // N_TILES
    t_loc = tile_id % N_TILES                # tile within core 0..99
    batch_g = t_loc // TB
    # bank q = quarter q of every core's node slice, matching the
    # quarter-AllGather output layout: bank row = core*QROWS + local row
    bank_id = (src % SLICE_N) // QROWS
    idx_local = (src // SLICE_N) * QROWS + (src % SLICE_N) % QROWS
    order = np.lexsort((tile_id, bank_id, batch_g, core_id))
    src_s = src[order]
    dst_s = dst[order]
    tloc_s = t_loc[order]

    # per (core, batch, bank, tile-in-batch) counts
    tl_in_b = tloc_s % TB
    key = (((core_id[order] * N_BATCH + batch_g[order]) * N_BANKS
            + bank_id[order]) * TB + tl_in_b)
    cnt4 = np.bincount(key, minlength=N_CORES * N_BATCH * N_BANKS * TB)
    cnt4 = cnt4.reshape(N_CORES, N_BATCH, N_BANKS, TB)
    cnt3 = cnt4.sum(axis=3)                                # [C, B, K]
    nbk = (-(-cnt3 // CHUNK)).max(axis=0)                  # [B, K] chunks

    # chunk columns + per-tile chunk ranges (union over cores)
    col0 = np.zeros((N_BATCH, N_BANKS), np.int64)
    c = 0
    for b in range(N_BATCH):
        for k in range(N_BANKS):
            col0[b, k] = c
            c += int(nbk[b, k])
    total_chunks = c

    ends = np.cumsum(cnt4, axis=3)                         # [C,B,K,TB]
    starts = ends - cnt4
    lo = np.where(cnt4 > 0, starts // CHUNK, np.iinfo(np.int64).max)
    hi = np.where(cnt4 > 0, -(-ends // CHUNK), 0)
    lo = lo.min(axis=0)                                    # [B,K,TB]
    hi = hi.max(axis=0)

    # runs per (batch, tile): contiguous chunk ranges split to <= KM
    tile_runs = {}
    for b in range(N_BATCH):
        for tl in range(TB):
            runs = []
            for k in range(N_BANKS):
                l, h = int(lo[b, k, tl]), int(hi[b, k, tl])
                if h <= l:
                    continue
                base = int(col0[b, k])
                x = l
                while x < h:
                    r = min(KM, h - x)
                    runs.append((base + x, r))
                    x += r
            tile_runs[(b, tl)] = runs

    # gather call pieces: split each (batch, bank) into two equal halves so
    # the round-robin SWDGE queues see balanced generation work
    pieces = []                      # (b, k, row0, rows, gl, col_start, off16)
    batch_pieces = [[] for _ in range(N_BATCH)]
    s16 = 0
    for b in range(N_BATCH):
        tmp = []                     # (si, k, piece_idx) for interleave sort
        for k in range(N_BANKS):
            nb = int(nbk[b, k])
            rows = BANK
            nsplit = max(1, -(-nb // MAXC))
            g0 = 0
            for si in range(nsplit):
                gl = (nb - g0) // (nsplit - si)
                if gl == 0:
                    continue
                tmp.append((si, k, len(pieces)))
                pieces.append((b, k, 0, rows, gl,
                               int(col0[b, k]) + g0, s16))
                s16 += gl * 8
                g0 += gl
        # issue order (si, k): first slice of every bank before any second
        # slice, so all 4 SWDGE queues start draining immediately
        batch_pieces[b] = [pi for _, _, pi in sorted(tmp)]
    s_total = s16

    # per-core idx/seg arrays in chunk-column order
    per_core = []
    seg_all = ((dst_s % TILE_N) + PAR_OFF * (tloc_s % 2)).astype(np.float16)
    idx_all = idx_local[order].astype(np.int16)
    # group start offset per (core, batch, bank) in sorted edge order
    grp_sizes = cnt3.reshape(-1)
    grp_off = np.concatenate([[0], np.cumsum(grp_sizes)])
    for cidx in range(N_CORES):
        idx_mat = np.zeros((total_chunks, CHUNK), np.int16)
        seg_mat = np.full((total_chunks, CHUNK), SENT, np.float16)
        for b in range(N_BATCH):
            for k in range(N_BANKS):
                g = (cidx * N_BATCH + b) * N_BANKS + k
                o0, m = int(grp_off[g]), int(grp_sizes[g])
                nb = int(nbk[b, k])
                ii = np.zeros(nb * CHUNK, np.int16)
                ss = np.full(nb * CHUNK, SENT, np.float16)
                ii[:m] = idx_all[o0 : o0 + m]
                ss[:m] = seg_all[o0 : o0 + m]
                c0 = int(col0[b, k])
                idx_mat[c0 : c0 + nb] = ii.reshape(nb, CHUNK)
                seg_mat[c0 : c0 + nb] = ss.reshape(nb, CHUNK)
        per_core.append((idx_mat, seg_mat))

    meta = {
        "total_chunks": total_chunks, "s_total": s_total,
        "pieces": pieces, "batch_pieces": batch_pieces,
        "tile_runs": tile_runs,
    }
    return per_core, meta, dinv, sqdeg


def _pack_idx(idx_mat, meta):
    """Wrap chunk-major indices into the dma_gather [16, n/16] layout per
    (batch, bank) block, concatenated, replicated to 128 partitions."""
    blocks = []
    for b, k, row0, rows, gl, cs, off16 in meta["pieces"]:
        flat = idx_mat[cs : cs + gl].reshape(-1)           # [gl*128]
        blocks.append(flat.reshape(-1, 16).T)              # [16, gl*8]
    packed = np.concatenate(blocks, axis=1)
    assert packed.shape[1] == meta["s_total"]
    return np.tile(packed, (8, 1)).copy()                  # [128, S]


def _pack_dinv(v, slice0, dtype):
    """[128, n_tiles]: partition p, col t = v[slice0 + t*TILE_N + p]."""
    out = np.zeros((128, N_TILES), dtype)
    sl = v[slice0 : slice0 + N_TILES * TILE_N].reshape(N_TILES, TILE_N)
    out[:TILE_N, :] = sl.T
    return out


# ---------------------------------------------------------------------------
# Device kernel builder
# ---------------------------------------------------------------------------

def _build(meta, has_bias):
    total_chunks = meta["total_chunks"]
    s_total = meta["s_total"]
    pieces = meta["pieces"]
    batch_pieces = meta["batch_pieces"]
    tile_runs = meta["tile_runs"]
    n_a_tiles = math.ceil(SLICE_N / 128)

    nc = bacc.Bacc("TRN2", target_bir_lowering=False, debug=False,
                   num_devices=N_CORES, num_swdge_queues=4,
                   dynamic_dma_scratch_size=16384)

    # inputs
    xsT = nc.dram_tensor("xsT", [D, SLICE_N], FP16, kind="ExternalInput")
    w1 = nc.dram_tensor("w1", [D, D], FP16, kind="ExternalInput")
    w2 = nc.dram_tensor("w2", [D, D], FP16, kind="ExternalInput")
    wfc = nc.dram_tensor("wfc", [D, DOUT], FP16, kind="ExternalInput")
    iota_in = nc.dram_tensor("iota", [128, 2 * KM * OHW], FP16,
                             kind="ExternalInput")
    dinvp = nc.dram_tensor("dinvp", [128, N_TILES], FP32, kind="ExternalInput")
    dinv2p = nc.dram_tensor("dinv2p", [128, N_TILES], FP32,
                            kind="ExternalInput")
    idx1 = nc.dram_tensor("idx1", [128, s_total], I16, kind="ExternalInput")
    seg1 = nc.dram_tensor("seg1", [128, 2 * total_chunks], FP16,
                          kind="ExternalInput")
    brows = (nc.dram_tensor("brows", [4, D], FP16, kind="ExternalInput")
             if has_bias else None)
    sqdegp = (nc.dram_tensor("sqdegp", [1, SLICE_N], FP16,
                             kind="ExternalInput") if has_bias else None)

    out = nc.dram_tensor("out", [SLICE_N, DOUT], FP32, kind="ExternalOutput")

    # internal dram: per-quarter bounce + gather-table tensors so each
    # quarter AllGather and its consumers have clean tensor-level deps
    g1_bounces = [nc.dram_tensor(f"g1_bounce{q}", [QROWS, D], FP16)
                  for q in range(N_BANKS)]
    g2_bounces = [nc.dram_tensor(f"g2_bounce{q}", [QROWS, D], FP16)
                  for q in range(N_BANKS)]
    g1_tables = [nc.dram_tensor(f"g1_table{q}", [BANK, D], FP16,
                                addr_space="Shared") for q in range(N_BANKS)]
    g2_tables = [nc.dram_tensor(f"g2_table{q}", [BANK, D], FP16,
                                addr_space="Shared") for q in range(N_BANKS)]

    warm_in = nc.dram_tensor("warm_in", [1, 16], FP16)
    warm_out = nc.dram_tensor("warm_out", [N_CORES, 16], FP16,
                              addr_space="Shared")

    with tile.TileContext(nc) as tc:
        with (
            tc.tile_pool(name="const", bufs=1) as constp,
            tc.tile_pool(name="aio", bufs=4) as aio,
            tc.tile_pool(name="msg", bufs=50) as msgp,
            tc.tile_pool(name="mp", bufs=7) as mp,
            tc.tile_pool(name="gs", bufs=4) as gsp,
            tc.tile_pool(name="fl", bufs=4) as flp,
            tc.tile_pool(name="lg", bufs=12) as lgp,
            tc.tile_pool(name="nm", bufs=12) as nmp,
            tc.tile_pool(name="acc", bufs=4, space="PSUM") as accp,
            tc.tile_pool(name="tps", bufs=2, space="PSUM") as tpsp,
            tc.tile_pool(name="gps", bufs=2, space="PSUM") as gpsp,
        ):
            # a minimal first collective: the runtime attaches its ~40-50us
            # init BARRIER to the first cc op, so issue one with no deps and
            # let it overlap the constant loads + stage A compute
            nc.gpsimd.collective_compute(
                "AllGather", mybir.AluOpType.bypass,
                ins=[warm_in[:, :]], outs=[warm_out[:, :]],
                replica_groups=[list(range(N_CORES))],
            )
            # constants
            w1_t = constp.tile([D, D], FP16, tag="w1")
            nc.sync.dma_start(out=w1_t[:], in_=w1[:, :])
            w2_t = constp.tile([D, D], FP16, tag="w2")
            nc.sync.dma_start(out=w2_t[:], in_=w2[:, :])
            wfc_t = constp.tile([D, DOUT], FP16, tag="wfc")
            nc.sync.dma_start(out=wfc_t[:], in_=wfc[:, :])
            iota_t = constp.tile([128, 2 * KM * OHW], FP16, tag="iota")
            nc.sync.dma_start(out=iota_t[:], in_=iota_in[:, :])
            dinv_t = constp.tile([128, N_TILES], FP32, tag="dinvp")
            nc.sync.dma_start(out=dinv_t[:], in_=dinvp[:, :])
            dinv2_t = constp.tile([128, N_TILES], FP32, tag="dinv2p")
            nc.sync.dma_start(out=dinv2_t[:], in_=dinv2p[:, :])
            idx_t = constp.tile([128, s_total], I16, tag="idx")
            nc.sync.dma_start(out=idx_t[:], in_=idx1[:, :])
            seg_t = constp.tile([128, 2 * total_chunks], FP16, tag="seg")
            nc.sync.dma_start(out=seg_t[:], in_=seg1[:, :])
            if has_bias:
                brow_ts = []
                for r in range(4):
                    bt = constp.tile([1, D], FP16, tag=f"brow{r}")
                    nc.sync.dma_start(out=bt[:], in_=brows[r : r + 1, :])
                    brow_ts.append(bt)
                sqdeg_t = constp.tile([1, SLICE_N], FP16, tag="sqdegp")
                nc.sync.dma_start(out=sqdeg_t[:], in_=sqdegp[:, :])
            ident_t = constp.tile([128, 128], FP16, tag="ident")
            from concourse.masks import make_identity
            make_identity(nc, ident_t[:])

            def emit_ag(bounce, table):
                nc.gpsimd.collective_compute(
                    "AllGather", mybir.AluOpType.bypass,
                    ins=[bounce[:, :]], outs=[table[:, :]],
                    replica_groups=[list(range(N_CORES))],
                )

            # ---------------- Stage A ----------------
            # per AG quarter: compute g1 rows, then AllGather that quarter
            # immediately so layer-1 gathers on bank q start ~3 quarters
            # earlier than a monolithic AllGather would allow
            BL = 4
            for q in range(N_BANKS):
                st = 0
                while st < QROWS:
                    bw = min(128 * BL, QROWS - st)
                    nch = math.ceil(bw / 128)
                    g0 = q * QROWS + st
                    xt = aio.tile([D, 128 * BL], FP16, tag="xt")
                    nc.sync.dma_start(out=xt[:, :bw], in_=xsT[:, g0 : g0 + bw])
                    gsb = aio.tile([128, BL, D], FP16, tag="gsb")
                    for i in range(nch):
                        w = min(128, bw - i * 128)
                        ps = accp.tile([128, D], FP32, tag="acc")
                        nc.tensor.matmul(out=ps[:w, :],
                                         lhsT=xt[:, i * 128 : i * 128 + w],
                                         rhs=w1_t[:], start=True, stop=True)
                        nc.vector.tensor_copy(out=gsb[:w, i, :], in_=ps[:w, :])
                    full = (bw // 128) * 128
                    if full:
                        nc.sync.dma_start(
                            out=g1_bounces[q][st : st + full, :]
                                .rearrange("(b p) d -> p b d", p=128),
                            in_=gsb[:, : full // 128, :])
                    if bw > full:
                        w = bw - full
                        nc.sync.dma_start(
                            out=g1_bounces[q][st + full : st + bw, :],
                            in_=gsb[:w, full // 128, :])
                    st += bw
                emit_ag(g1_bounces[q], g1_tables[q])

            # ---------------- aggregation layers ----------------
            qrr = [0]
            _regs = {}

            def nidx_reg(v):
                if v not in _regs:
                    _regs[v] = nc.gpsimd.to_reg(v)
                return _regs[v]

            def issue_piece(tables, pi, cmap):
                b, k, row0, rows, gl, cs, off16 = pieces[pi]
                mt = msgp.tile([128, gl, D], FP16, tag="msg")
                # queue = bank so a not-yet-AllGathered bank only stalls its
                # own SWDGE queue
                nc.gpsimd.dma_gather(
                    out_ap=mt[:],
                    in_ap=tables[k][row0 : row0 + rows, :],
                    idxs_ap=idx_t[:, off16 : off16 + gl * 8],
                    num_idxs=gl * CHUNK,
                    num_idxs_reg=nidx_reg(gl * CHUNK),
                    elem_size=D,
                    single_packet=False,
                    queue_num=k,
                )
                for j in range(gl):
                    cmap[cs + j] = (mt, j)

            def issue_batch(tables, bi):
                cmap = {}
                for pi in batch_pieces[bi]:
                    issue_piece(tables, pi, cmap)
                return cmap

            def process_batch(bi, cmap, bounces, brow_idx, flush):
                for tl in range(TB):
                    tg = bi * TB + tl
                    t0 = (tg % TPQ) * TILE_N
                    acc = accp.tile([128, D], FP32, tag="acc")
                    # self-loop: identity @ g_local (flush's *dinv[d] covers
                    # the dinv[d]^2 * h[d] = dinv[d] * g[d] self message)
                    gself = gsp.tile([128, D], FP16, tag="gself")
                    nc.sync.dma_start(out=gself[:TILE_N, :],
                                      in_=bounces[tg // TPQ]
                                          [t0 : t0 + TILE_N, :])
                    nc.tensor.matmul(out=acc[:TILE_N, :],
                                     lhsT=ident_t[:TILE_N, :TILE_N],
                                     rhs=gself[:TILE_N, :],
                                     start=True, stop=False)
                    if has_bias:
                        nc.tensor.matmul(
                            out=acc[:TILE_N, :],
                            lhsT=sqdeg_t[:, tg * TILE_N : tg * TILE_N + TILE_N],
                            rhs=brow_ts[brow_idx][:, :],
                            start=False, stop=False,
                        )
                    runs = tile_runs[(bi, tl)]
                    nch = sum(r for _, r in runs)
                    par = tl % 2
                    io0 = par * KM * OHW
                    ci = 0
                    for col0, r in runs:
                        # one-hot [128, r, 126] via one tensor_tensor
                        # per run.  seg is stored duplicated in PAIRS so the
                        # innermost AP dim of every operand is packed stride-1
                        # fp16 -> DVE 2x_1p mode (a plain seg broadcast has
                        # inner stride 0 and falls back to 1x).  Col 125 of
                        # each one-hot row compares against a 999 pad value
                        # (never matches); the matmul uses cols 0..124.
                        mtile = mp.tile([128, KM, OHW], FP16, tag="m")
                        nc.vector.tensor_tensor(
                            out=mtile[:, :r, :]
                                .rearrange("p r (x2 xi) -> p r x2 xi", xi=2),
                            in0=seg_t[:, 2 * col0 : 2 * (col0 + r)]
                                .rearrange("p (r o xi) -> p r o xi",
                                           o=1, xi=2)
                                .to_broadcast([128, r, OHW // 2, 2]),
                            in1=iota_t[:, io0 : io0 + r * OHW]
                                .rearrange("p (r x2 xi) -> p r x2 xi",
                                           r=r, xi=2),
                            op=mybir.AluOpType.is_equal,
                        )
                        for i in range(r):
                            mt, j = cmap[col0 + i]
                            ci += 1
                            nc.tensor.matmul(
                                out=acc[:TILE_N, :],
                                lhsT=mtile[:, i, :TILE_N],
                                rhs=mt[:, j, :],
                                start=False,
                                stop=(ci == nch),
                            )
                    flush(tg, acc)

            def agg_layer(tables, bounces, brow_idx, flush,
                          post_batch=None, ag_hook=None):
                issued = {}
                for bi in range(min(AHEAD + 1, N_BATCH)):
                    issued[bi] = issue_batch(tables, bi)
                for bi in range(N_BATCH):
                    process_batch(bi, issued.pop(bi), bounces, brow_idx, flush)
                    if post_batch is not None:
                        post_batch()
                    if ag_hook is not None:
                        ag_hook(bi)
                    nxt = bi + AHEAD + 1
                    if nxt < N_BATCH:
                        issued[nxt] = issue_batch(tables, nxt)

            # Flush B: s = dinv*relu(dinv*acc) = relu(dinv^2*acc);
            # g2 = s @ W2
            def flush_b(tg, acc):
                dv2 = dinv2_t[:TILE_N, tg : tg + 1]
                s = flp.tile([128, D], FP16, tag="s")
                nc.scalar.activation(out=s[:TILE_N, :], in_=acc[:TILE_N, :],
                                     func=mybir.ActivationFunctionType.Relu,
                                     scale=dv2)
                stp = tpsp.tile([128, TILE_N], FP16, tag="stp")
                nc.tensor.transpose(out=stp[:], in_=s[:TILE_N, :],
                                    identity=ident_t[:TILE_N, :TILE_N])
                stb = flp.tile([128, TILE_N], FP16, tag="stb")
                nc.vector.tensor_copy(out=stb[:], in_=stp[:])
                g2p = gpsp.tile([128, D], FP32, tag="g2p")
                nc.tensor.matmul(out=g2p[:TILE_N, :], lhsT=stb[:],
                                 rhs=w2_t[:], start=True, stop=True)
                g2sb = flp.tile([128, D], FP16, tag="g2sb")
                nc.vector.tensor_copy(out=g2sb[:TILE_N, :], in_=g2p[:TILE_N, :])
                o0 = (tg % TPQ) * TILE_N
                nc.scalar.dma_start(
                    out=g2_bounces[tg // TPQ][o0 : o0 + TILE_N, :],
                    in_=g2sb[:TILE_N, :],
                )

            # Flush C: h2 = relu(dinv*acc); logits -> SBUF; max; Exp inline
            # with accum_out into a per-batch [128, TB] sum tile.  A SINGLE
            # Ln per batch then consumes all TB sums (the Tile scheduler
            # would otherwise interleave per-tile Exp/Ln and thrash the
            # activation-table loads, ~1.3us each).
            c_pend = []
            c_state = {"bsum": None}

            def flush_c(tg, acc):
                dv = dinv_t[:TILE_N, tg : tg + 1]
                h2 = flp.tile([128, D], FP16, tag="h1")
                nc.scalar.activation(out=h2[:TILE_N, :], in_=acc[:TILE_N, :],
                                     func=mybir.ActivationFunctionType.Relu,
                                     scale=dv)
                htp = tpsp.tile([128, TILE_N], FP16, tag="stp")
                nc.tensor.transpose(out=htp[:], in_=h2[:TILE_N, :],
                                    identity=ident_t[:TILE_N, :TILE_N])
                htb = flp.tile([128, TILE_N], FP16, tag="stb")
                nc.vector.tensor_copy(out=htb[:], in_=htp[:])
                lg = gpsp.tile([128, DOUT], FP32, tag="g2p")
                nc.tensor.matmul(out=lg[:TILE_N, :], lhsT=htb[:],
                                 rhs=wfc_t[:], start=True, stop=not has_bias)
                if has_bias:
                    nc.tensor.matmul(out=lg[:TILE_N, :],
                                     lhsT=brow_ts[3][:, :TILE_N],
                                     rhs=brow_ts[2][:, :DOUT],
                                     start=False, stop=True)
                lgs = lgp.tile([128, DOUT], FP32, tag="lgs")
                nc.vector.tensor_copy(out=lgs[:TILE_N, :], in_=lg[:TILE_N, :])
                mx = flp.tile([128, 1], FP32, tag="mx")
                nc.vector.tensor_reduce(out=mx[:TILE_N, :], in_=lg[:TILE_N, :],
                                        axis=mybir.AxisListType.X,
                                        op=mybir.AluOpType.max)
                negm = nmp.tile([128, 1], FP32, tag="negm")
                nc.vector.tensor_scalar_mul(out=negm[:TILE_N, :],
                                            in0=mx[:TILE_N, :], scalar1=-1.0)
                if c_state["bsum"] is None:
                    bsum_t = nmp.tile([128, TB], FP32, tag="bsum")
                    c_state["bsum"] = bsum_t
                slot = len(c_pend)
                esc = flp.tile([128, DOUT], FP16, tag="esc")
                nc.scalar.activation(out=esc[:TILE_N, :],
                                     in_=lgs[:TILE_N, :],
                                     func=mybir.ActivationFunctionType.Exp,
                                     bias=negm[:TILE_N, :],
                                     accum_out=c_state["bsum"][:TILE_N,
                                                              slot : slot + 1])
                c_pend.append((tg, lgs, negm, slot))

            def post_batch_c():
                bsum = c_state["bsum"]
                nb = len(c_pend)
                lns = flp.tile([128, TB], FP32, tag="lns")
                nc.scalar.activation(out=lns[:TILE_N, :nb],
                                     in_=bsum[:TILE_N, :nb],
                                     func=mybir.ActivationFunctionType.Ln)
                for tg, lgs, negm, slot in c_pend:
                    nmls = flp.tile([128, 1], FP32, tag="nmls")
                    nc.vector.tensor_tensor(out=nmls[:TILE_N, :],
                                            in0=negm[:TILE_N, :],
                                            in1=lns[:TILE_N, slot : slot + 1],
                                            op=mybir.AluOpType.subtract)
                    ot = flp.tile([128, DOUT], FP32, tag="ot")
                    nc.vector.tensor_tensor(out=ot[:TILE_N, :],
                                            in0=lgs[:TILE_N, :],
                                            in1=nmls[:TILE_N, :]
                                                .to_broadcast([TILE_N, DOUT]),
                                            op=mybir.AluOpType.add)
                    nc.sync.dma_start(
                        out=out[tg * TILE_N : (tg + 1) * TILE_N, :],
                        in_=ot[:TILE_N, :],
                    )
                c_pend.clear()
                c_state["bsum"] = None

            ag2_done = set()

            def ag2_hook(bi):
                tiles_done = (bi + 1) * TB
                for q in range(N_BANKS):
                    if q not in ag2_done and tiles_done >= (q + 1) * TPQ:
                        emit_ag(g2_bounces[q], g2_tables[q])
                        ag2_done.add(q)

            agg_layer(g1_tables, g1_bounces, 0, flush_b, ag_hook=ag2_hook)
            agg_layer(g2_tables, g2_bounces, 1, flush_c, post_batch_c)

    nc.compile()
    return nc


# ---------------------------------------------------------------------------
# Public entry point
# ---------------------------------------------------------------------------

_CACHE = {}


def kernel(x, edge_index, W1, b1, W2, b2, Wfc, bfc):
    x = np.asarray(x, np.float32)
    per_core, meta, dinv, sqdeg = _preprocess(edge_index)

    has_bias = bool(
        np.any(np.asarray(b1)) or np.any(np.asarray(b2)) or np.any(np.asarray(bfc))
    )
    mkey = hashlib.sha1(
        repr((meta["total_chunks"], meta["s_total"], meta["pieces"],
              sorted(meta["tile_runs"].items()), has_bias)).encode()
    ).hexdigest()
    if mkey not in _CACHE:
        _CACHE[mkey] = _build(meta, has_bias)
    nc = _CACHE[mkey]

    xs = (dinv[:, None] * x).T           # [D, n]
    xsT_h = np.ascontiguousarray(xs).astype(np.float16)
    # iota: KM repeats of [0..124, pad] (+PAR_OFF for odd tiles); the
    # pad col value 999 matches neither parity's seg range nor SENT
    row = np.concatenate([np.arange(TILE_N, dtype=np.float32), [999.0]])
    base = np.tile(row, KM)
    iota = np.concatenate([base, base + PAR_OFF]).astype(np.float16)
    iota = np.tile(iota, (128, 1))
    if has_bias:
        brows_np = np.zeros((4, D), np.float32)
        brows_np[0, :] = np.asarray(b1, np.float32)
        brows_np[1, :] = np.asarray(b2, np.float32)
        brows_np[2, : DOUT] = np.asarray(bfc, np.float32)
        brows_np[3, :] = 1.0
        brows_np = brows_np.astype(np.float16)

    in_maps = []
    for c in range(N_CORES):
        s0 = c * SLICE_N
        idx_mat, seg_mat = per_core[c]
        im = {
            "xsT": np.ascontiguousarray(xsT_h[:, s0 : s0 + SLICE_N]),
            "w1": np.asarray(W1, np.float32).astype(np.float16),
            "w2": np.asarray(W2, np.float32).astype(np.float16),
            "wfc": np.asarray(Wfc, np.float32).astype(np.float16),
            "iota": iota,
            "dinvp": _pack_dinv(dinv, s0, np.float32),
            "dinv2p": _pack_dinv(dinv * dinv, s0, np.float32),
            "idx1": _pack_idx(idx_mat, meta),
            "seg1": np.ascontiguousarray(np.repeat(seg_mat.T, 2, axis=1)),
        }
        if has_bias:
            im["brows"] = brows_np
            im["sqdegp"] = sqdeg[s0 : s0 + SLICE_N][None, :].astype(np.float16)
        in_maps.append(im)

    global _last_in_maps
    _last_in_maps = in_maps
    last_exc = None
    for _attempt in range(3):
        try:
            res = bass_utils.run_bass_kernel_spmd(
                nc, in_maps, core_ids=list(range(N_CORES))
            )
            return np.concatenate(
                [res.results[c]["out"] for c in range(N_CORES)], axis=0
            )
        except Exception as e:  # transient device/tunnel errors: retry
            last_exc = e
    raise last_exc


_last_in_maps = None



# revision 15
# speedup vs baseline: 1.0382x; 1.0382x over previous
"""Self-contained Trainium2 Bass kernel for a 2-layer GCN + FC + log_softmax.

Distribution: nodes sharded across 8 NeuronCores (12500 rows each); edges
partitioned by destination node so each core's scatter-add is local; the
per-layer "gather tables" g = D^-1/2 * H * W are exchanged with an on-chip
AllGather; small weights replicated.

Device algorithm per core:
  Stage A : g1 slice = (dinv*x) @ W1 (rows of this core), fp16 -> AllGather
  Agg     : per 125-node dst tile: PSUM += onehot(seg).T @ g1[src]
            (dma_gather of fp16 rows from 4 HBM banks on 4 SWDGE queues,
            one-hot built on VectorE from preloaded seg values, segment-sum
            as TensorE matmul).  Self-loops are NOT gathered: they are a
            diag(dinv) matmul against the core-local g rows.
  Flush B : h1 = relu(dinv*acc); g2 = (dinv*h1) @ W2 -> AllGather
  Flush C : h2 = relu(dinv*acc); logits = h2 @ Wfc; fused log_softmax.
"""
import hashlib
import math

import numpy as np
import ml_dtypes

import concourse.bass as bass
import concourse.mybir as mybir
import concourse.tile as tile
from concourse import bacc, bass_utils

FP16_NP = ml_dtypes.float16 if hasattr(ml_dtypes, "float16") else np.float16

# Problem contract (hardcoded; must match setup_inputs()).
N_NODES = 100000
N_EDGES = 1600000
D = 128
DOUT = 40

N_CORES = 8
TILE_N = 125            # dst nodes per PSUM tile
TB = 10                 # dst tiles per batch
BANK = 25000            # gather table bank rows (int16 index limit 32767)
HALF = 6250             # node rows per core per table half (AG split)
CHUNK = 128             # edges per matmul chunk
MAXC = 12               # chunks per dma_gather call
KM = 8                  # max chunks per one-hot build run
AHEAD = 2               # gather issue lookahead (batches)
PAR_OFF = 500.0         # seg offset for odd tiles (collision guard)
SENT = 1250.0           # seg sentinel (matches no iota value)
OHW = 126               # one-hot row width (125 cols + 1 pad col)

FP32 = mybir.dt.float32
FP16 = mybir.dt.float16
I16 = mybir.dt.int16

SLICE_N = N_NODES // N_CORES            # 12500
QROWS = SLICE_N // 4                    # local rows per AG quarter
N_TILES = SLICE_N // TILE_N             # 100
N_BATCH = N_TILES // TB                 # 10
N_BANKS = (N_NODES + BANK - 1) // BANK  # 4
TPQ = 25                                # dst tiles per bounce quarter


# ---------------------------------------------------------------------------
# Host preprocessing
# ---------------------------------------------------------------------------

def _preprocess(edge_index):
    """Sort edges by (core, batch, bank, tile); build the static chunk
    structure shared by both conv layers plus per-core idx/seg arrays."""
    ei = np.asarray(edge_index, np.int64)
    src, dst = ei[0], ei[1]
    deg = (np.bincount(dst, minlength=N_NODES) + 1.0).astype(np.float32)
    dinv = 1.0 / np.sqrt(deg)
    sqdeg = np.sqrt(deg)

    tile_id = dst // TILE_N                  # global tile 0..799
    core_id = tile_id // N_TILES
    t_loc = tile_id % N_TILES                # tile within core 0..99
    batch_g = t_loc // TB
    # bank q = quarter q of every core's node slice, matching the
    # quarter-AllGather output layout: bank row = core*QROWS + local row
    bank_id = (src % SLICE_N) // QROWS
    idx_local = (src // SLICE_N) * QROWS + (src % SLICE_N) % QROWS
    order = np.lexsort((tile_id, bank_id, batch_g, core_id))
    src_s = src[order]
    dst_s = dst[order]
    tloc_s = t_loc[order]

    # per (core, batch, bank, tile-in-batch) counts
    tl_in_b = tloc_s % TB
    key = (((core_id[order] * N_BATCH + batch_g[order]) * N_BANKS
            + bank_id[order]) * TB + tl_in_b)
    cnt4 = np.bincount(key, minlength=N_CORES * N_BATCH * N_BANKS * TB)
    cnt4 = cnt4.reshape(N_CORES, N_BATCH, N_BANKS, TB)
    cnt3 = cnt4.sum(axis=3)                                # [C, B, K]
    nbk = (-(-cnt3 // CHUNK)).max(axis=0)                  # [B, K] chunks

    # chunk columns + per-tile chunk ranges (union over cores)
    col0 = np.zeros((N_BATCH, N_BANKS), np.int64)
    c = 0
    for b in range(N_BATCH):
        for k in range(N_BANKS):
            col0[b, k] = c
            c += int(nbk[b, k])
    total_chunks = c

    ends = np.cumsum(cnt4, axis=3)                         # [C,B,K,TB]
    starts = ends - cnt4
    lo = np.where(cnt4 > 0, starts // CHUNK, np.iinfo(np.int64).max)
    hi = np.where(cnt4 > 0, -(-ends // CHUNK), 0)
    lo = lo.min(axis=0)                                    # [B,K,TB]
    hi = hi.max(axis=0)

    # runs per (batch, tile): contiguous chunk ranges split to <= KM
    tile_runs = {}
    for b in range(N_BATCH):
        for tl in range(TB):
            runs = []
            for k in range(N_BANKS):
                l, h = int(lo[b, k, tl]), int(hi[b, k, tl])
                if h <= l:
                    continue
                base = int(col0[b, k])
                x = l
                while x < h:
                    r = min(KM, h - x)
                    runs.append((base + x, r))
                    x += r
            tile_runs[(b, tl)] = runs

    # gather call pieces: split each (batch, bank) into two equal halves so
    # the round-robin SWDGE queues see balanced generation work
    pieces = []                      # (b, k, row0, rows, gl, col_start, off16)
    batch_pieces = [[] for _ in range(N_BATCH)]
    s16 = 0
    for b in range(N_BATCH):
        tmp = []                     # (si, k, piece_idx) for interleave sort
        for k in range(N_BANKS):
            nb = int(nbk[b, k])
            rows = BANK
            nsplit = max(1, -(-nb // MAXC))
            g0 = 0
            for si in range(nsplit):
                gl = (nb - g0) // (nsplit - si)
                if gl == 0:
                    continue
                tmp.append((si, k, len(pieces)))
                pieces.append((b, k, 0, rows, gl,
                               int(col0[b, k]) + g0, s16))
                s16 += gl * 8
                g0 += gl
        # issue order (si, k): first slice of every bank before any second
        # slice, so all 4 SWDGE queues start draining immediately
        batch_pieces[b] = [pi for _, _, pi in sorted(tmp)]
    s_total = s16

    # per-core idx/seg arrays in chunk-column order
    per_core = []
    seg_all = ((dst_s % TILE_N) + PAR_OFF * (tloc_s % 2)).astype(np.float16)
    idx_all = idx_local[order].astype(np.int16)
    # group start offset per (core, batch, bank) in sorted edge order
    grp_sizes = cnt3.reshape(-1)
    grp_off = np.concatenate([[0], np.cumsum(grp_sizes)])
    for cidx in range(N_CORES):
        idx_mat = np.zeros((total_chunks, CHUNK), np.int16)
        seg_mat = np.full((total_chunks, CHUNK), SENT, np.float16)
        for b in range(N_BATCH):
            for k in range(N_BANKS):
                g = (cidx * N_BATCH + b) * N_BANKS + k
                o0, m = int(grp_off[g]), int(grp_sizes[g])
                nb = int(nbk[b, k])
                ii = np.zeros(nb * CHUNK, np.int16)
                ss = np.full(nb * CHUNK, SENT, np.float16)
                ii[:m] = idx_all[o0 : o0 + m]
                ss[:m] = seg_all[o0 : o0 + m]
                c0 = int(col0[b, k])
                idx_mat[c0 : c0 + nb] = ii.reshape(nb, CHUNK)
                seg_mat[c0 : c0 + nb] = ss.reshape(nb, CHUNK)
        per_core.append((idx_mat, seg_mat))

    meta = {
        "total_chunks": total_chunks, "s_total": s_total,
        "pieces": pieces, "batch_pieces": batch_pieces,
        "tile_runs": tile_runs,
    }
    return per_core, meta, dinv, sqdeg


def _pack_idx(idx_mat, meta):
    """Wrap chunk-major indices into the dma_gather [16, n/16] layout per
    (batch, bank) block, concatenated, replicated to 128 partitions."""
    blocks = []
    for b, k, row0, rows, gl, cs, off16 in meta["pieces"]:
        flat = idx_mat[cs : cs + gl].reshape(-1)           # [gl*128]
        blocks.append(flat.reshape(-1, 16).T)              # [16, gl*8]
    packed = np.concatenate(blocks, axis=1)
    assert packed.shape[1] == meta["s_total"]
    return np.tile(packed, (8, 1)).copy()                  # [128, S]


def _pack_dinv(v, slice0, dtype):
    """[128, n_tiles]: partition p, col t = v[slice0 + t*TILE_N + p]."""
    out = np.zeros((128, N_TILES), dtype)
    sl = v[slice0 : slice0 + N_TILES * TILE_N].reshape(N_TILES, TILE_N)
    out[:TILE_N, :] = sl.T
    return out


# ---------------------------------------------------------------------------
# Device kernel builder
# ---------------------------------------------------------------------------

def _build(meta, has_bias):
    total_chunks = meta["total_chunks"]
    s_total = meta["s_total"]
    pieces = meta["pieces"]
    batch_pieces = meta["batch_pieces"]
    tile_runs = meta["tile_runs"]
    n_a_tiles = math.ceil(SLICE_N / 128)

    nc = bacc.Bacc("TRN2", target_bir_lowering=False, debug=False,
                   num_devices=N_CORES, num_swdge_queues=4,
                   dynamic_dma_scratch_size=16384)

    # inputs
    xsT = nc.dram_tensor("xsT", [D, SLICE_N], FP16, kind="ExternalInput")
    w1 = nc.dram_tensor("w1", [D, D], FP16, kind="ExternalInput")
    w2 = nc.dram_tensor("w2", [D, D], FP16, kind="ExternalInput")
    wfc = nc.dram_tensor("wfc", [D, DOUT], FP16, kind="ExternalInput")
    iota_in = nc.dram_tensor("iota", [128, 2 * KM * OHW], FP16,
                             kind="ExternalInput")
    dinvp = nc.dram_tensor("dinvp", [128, N_TILES], FP32, kind="ExternalInput")
    dinv2p = nc.dram_tensor("dinv2p", [128, N_TILES], FP32,
                            kind="ExternalInput")
    idx1 = nc.dram_tensor("idx1", [128, s_total], I16, kind="ExternalInput")
    seg1 = nc.dram_tensor("seg1", [128, 2 * total_chunks], FP16,
                          kind="ExternalInput")
    brows = (nc.dram_tensor("brows", [4, D], FP16, kind="ExternalInput")
             if has_bias else None)
    sqdegp = (nc.dram_tensor("sqdegp", [1, SLICE_N], FP16,
                             kind="ExternalInput") if has_bias else None)

    out = nc.dram_tensor("out", [SLICE_N, DOUT], FP32, kind="ExternalOutput")

    # internal dram: per-quarter bounce + gather-table tensors so each
    # quarter AllGather and its consumers have clean tensor-level deps
    g1_bounces = [nc.dram_tensor(f"g1_bounce{q}", [QROWS, D], FP16)
                  for q in range(N_BANKS)]
    g2_bounces = [nc.dram_tensor(f"g2_bounce{q}", [QROWS, D], FP16)
                  for q in range(N_BANKS)]
    g1_tables = [nc.dram_tensor(f"g1_table{q}", [BANK, D], FP16,
                                addr_space="Shared") for q in range(N_BANKS)]
    g2_tables = [nc.dram_tensor(f"g2_table{q}", [BANK, D], FP16,
                                addr_space="Shared") for q in range(N_BANKS)]

    warm_in = nc.dram_tensor("warm_in", [1, 16], FP16)
    warm_out = nc.dram_tensor("warm_out", [N_CORES, 16], FP16,
                              addr_space="Shared")

    with tile.TileContext(nc) as tc:
        with (
            tc.tile_pool(name="const", bufs=1) as constp,
            tc.tile_pool(name="aio", bufs=4) as aio,
            tc.tile_pool(name="msg", bufs=48) as msgp,
            tc.tile_pool(name="mp", bufs=6) as mp,
            tc.tile_pool(name="gs", bufs=4) as gsp,
            tc.tile_pool(name="fl", bufs=4) as flp,
            tc.tile_pool(name="lg", bufs=12) as lgp,
            tc.tile_pool(name="nm", bufs=12) as nmp,
            tc.tile_pool(name="acc", bufs=4, space="PSUM") as accp,
            tc.tile_pool(name="tps", bufs=2, space="PSUM") as tpsp,
            tc.tile_pool(name="gps", bufs=2, space="PSUM") as gpsp,
        ):
            # a minimal first collective: the runtime attaches its ~40-50us
            # init BARRIER to the first cc op, so issue one with no deps and
            # let it overlap the constant loads + stage A compute
            nc.gpsimd.collective_compute(
                "AllGather", mybir.AluOpType.bypass,
                ins=[warm_in[:, :]], outs=[warm_out[:, :]],
                replica_groups=[list(range(N_CORES))],
            )
            # constants
            w1_t = constp.tile([D, D], FP16, tag="w1")
            nc.sync.dma_start(out=w1_t[:], in_=w1[:, :])
            w2_t = constp.tile([D, D], FP16, tag="w2")
            nc.sync.dma_start(out=w2_t[:], in_=w2[:, :])
            wfc_t = constp.tile([D, DOUT], FP16, tag="wfc")
            nc.sync.dma_start(out=wfc_t[:], in_=wfc[:, :])
            iota_t = constp.tile([128, 2 * KM * OHW], FP16, tag="iota")
            nc.sync.dma_start(out=iota_t[:], in_=iota_in[:, :])
            dinv_t = constp.tile([128, N_TILES], FP32, tag="dinvp")
            nc.sync.dma_start(out=dinv_t[:], in_=dinvp[:, :])
            dinv2_t = constp.tile([128, N_TILES], FP32, tag="dinv2p")
            nc.sync.dma_start(out=dinv2_t[:], in_=dinv2p[:, :])
            idx_t = constp.tile([128, s_total], I16, tag="idx")
            nc.sync.dma_start(out=idx_t[:], in_=idx1[:, :])
            seg_t = constp.tile([128, 2 * total_chunks], FP16, tag="seg")
            nc.sync.dma_start(out=seg_t[:], in_=seg1[:, :])
            if has_bias:
                brow_ts = []
                for r in range(4):
                    bt = constp.tile([1, D], FP16, tag=f"brow{r}")
                    nc.sync.dma_start(out=bt[:], in_=brows[r : r + 1, :])
                    brow_ts.append(bt)
                sqdeg_t = constp.tile([1, SLICE_N], FP16, tag="sqdegp")
                nc.sync.dma_start(out=sqdeg_t[:], in_=sqdegp[:, :])
            ident_t = constp.tile([128, 128], FP16, tag="ident")
            from concourse.masks import make_identity
            make_identity(nc, ident_t[:])

            def emit_ag(bounce, table):
                nc.gpsimd.collective_compute(
                    "AllGather", mybir.AluOpType.bypass,
                    ins=[bounce[:, :]], outs=[table[:, :]],
                    replica_groups=[list(range(N_CORES))],
                )

            # ---------------- Stage A ----------------
            # per AG quarter: compute g1 rows, then AllGather that quarter
            # immediately so layer-1 gathers on bank q start ~3 quarters
            # earlier than a monolithic AllGather would allow
            BL = 4
            for q in range(N_BANKS):
                st = 0
                while st < QROWS:
                    bw = min(128 * BL, QROWS - st)
                    nch = math.ceil(bw / 128)
                    g0 = q * QROWS + st
                    xt = aio.tile([D, 128 * BL], FP16, tag="xt")
                    nc.sync.dma_start(out=xt[:, :bw], in_=xsT[:, g0 : g0 + bw])
                    gsb = aio.tile([128, BL, D], FP16, tag="gsb")
                    for i in range(nch):
                        w = min(128, bw - i * 128)
                        ps = accp.tile([128, D], FP32, tag="acc")
                        nc.tensor.matmul(out=ps[:w, :],
                                         lhsT=xt[:, i * 128 : i * 128 + w],
                                         rhs=w1_t[:], start=True, stop=True)
                        nc.vector.tensor_copy(out=gsb[:w, i, :], in_=ps[:w, :])
                    full = (bw // 128) * 128
                    if full:
                        nc.sync.dma_start(
                            out=g1_bounces[q][st : st + full, :]
                                .rearrange("(b p) d -> p b d", p=128),
                            in_=gsb[:, : full // 128, :])
                    if bw > full:
                        w = bw - full
                        nc.sync.dma_start(
                            out=g1_bounces[q][st + full : st + bw, :],
                            in_=gsb[:w, full // 128, :])
                    st += bw
                emit_ag(g1_bounces[q], g1_tables[q])

            # ---------------- aggregation layers ----------------
            qrr = [0]
            _regs = {}

            def nidx_reg(v):
                if v not in _regs:
                    _regs[v] = nc.gpsimd.to_reg(v)
                return _regs[v]

            def issue_piece(tables, pi, cmap):
                b, k, row0, rows, gl, cs, off16 = pieces[pi]
                mt = msgp.tile([128, gl, D], FP16, tag="msg")
                # queue = bank so a not-yet-AllGathered bank only stalls its
                # own SWDGE queue
                nc.gpsimd.dma_gather(
                    out_ap=mt[:],
                    in_ap=tables[k][row0 : row0 + rows, :],
                    idxs_ap=idx_t[:, off16 : off16 + gl * 8],
                    num_idxs=gl * CHUNK,
                    num_idxs_reg=nidx_reg(gl * CHUNK),
                    elem_size=D,
                    single_packet=False,
                    queue_num=k,
                )
                for j in range(gl):
                    cmap[cs + j] = (mt, j)

            def issue_batch(tables, bi):
                cmap = {}
                for pi in batch_pieces[bi]:
                    issue_piece(tables, pi, cmap)
                return cmap

            def process_batch(bi, cmap, bounces, brow_idx, flush):
                for tl in range(TB):
                    tg = bi * TB + tl
                    t0 = (tg % TPQ) * TILE_N
                    acc = accp.tile([128, D], FP32, tag="acc")
                    # self-loop: identity @ g_local (flush's *dinv[d] covers
                    # the dinv[d]^2 * h[d] = dinv[d] * g[d] self message)
                    gself = gsp.tile([128, D], FP16, tag="gself")
                    nc.sync.dma_start(out=gself[:TILE_N, :],
                                      in_=bounces[tg // TPQ]
                                          [t0 : t0 + TILE_N, :])
                    nc.tensor.matmul(out=acc[:TILE_N, :],
                                     lhsT=ident_t[:TILE_N, :TILE_N],
                                     rhs=gself[:TILE_N, :],
                                     start=True, stop=False)
                    if has_bias:
                        nc.tensor.matmul(
                            out=acc[:TILE_N, :],
                            lhsT=sqdeg_t[:, tg * TILE_N : tg * TILE_N + TILE_N],
                            rhs=brow_ts[brow_idx][:, :],
                            start=False, stop=False,
                        )
                    runs = tile_runs[(bi, tl)]
                    nch = sum(r for _, r in runs)
                    par = tl % 2
                    io0 = par * KM * OHW
                    ci = 0
                    for col0, r in runs:
                        # one-hot [128, r, 126] via one tensor_tensor
                        # per run.  seg is stored duplicated in PAIRS so the
                        # innermost AP dim of every operand is packed stride-1
                        # fp16 -> DVE 2x_1p mode (a plain seg broadcast has
                        # inner stride 0 and falls back to 1x).  Col 125 of
                        # each one-hot row compares against a 999 pad value
                        # (never matches); the matmul uses cols 0..124.
                        mtile = mp.tile([128, KM, OHW], FP16, tag="m")
                        nc.vector.tensor_tensor(
                            out=mtile[:, :r, :]
                                .rearrange("p r (x2 xi) -> p r x2 xi", xi=2),
                            in0=seg_t[:, 2 * col0 : 2 * (col0 + r)]
                                .rearrange("p (r o xi) -> p r o xi",
                                           o=1, xi=2)
                                .to_broadcast([128, r, OHW // 2, 2]),
                            in1=iota_t[:, io0 : io0 + r * OHW]
                                .rearrange("p (r x2 xi) -> p r x2 xi",
                                           r=r, xi=2),
                            op=mybir.AluOpType.is_equal,
                        )
                        for i in range(r):
                            mt, j = cmap[col0 + i]
                            ci += 1
                            nc.tensor.matmul(
                                out=acc[:TILE_N, :],
                                lhsT=mtile[:, i, :TILE_N],
                                rhs=mt[:, j, :],
                                start=False,
                                stop=(ci == nch),
                            )
                    flush(tg, acc)

            def agg_layer(tables, bounces, brow_idx, flush,
                          post_batch=None, ag_hook=None):
                issued = {}
                for bi in range(min(AHEAD + 1, N_BATCH)):
                    issued[bi] = issue_batch(tables, bi)
                for bi in range(N_BATCH):
                    process_batch(bi, issued.pop(bi), bounces, brow_idx, flush)
                    if post_batch is not None:
                        post_batch()
                    if ag_hook is not None:
                        ag_hook(bi)
                    nxt = bi + AHEAD + 1
                    if nxt < N_BATCH:
                        issued[nxt] = issue_batch(tables, nxt)

            # Flush B: s = dinv*relu(dinv*acc) = relu(dinv^2*acc);
            # g2 = s @ W2
            def flush_b(tg, acc):
                dv2 = dinv2_t[:TILE_N, tg : tg + 1]
                s = flp.tile([128, D], FP16, tag="s")
                nc.scalar.activation(out=s[:TILE_N, :], in_=acc[:TILE_N, :],
                                     func=mybir.ActivationFunctionType.Relu,
                                     scale=dv2)
                stp = tpsp.tile([128, TILE_N], FP16, tag="stp")
                nc.tensor.transpose(out=stp[:], in_=s[:TILE_N, :],
                                    identity=ident_t[:TILE_N, :TILE_N])
                stb = flp.tile([128, TILE_N], FP16, tag="stb")
                nc.vector.tensor_copy(out=stb[:], in_=stp[:])
                g2p = gpsp.tile([128, D], FP32, tag="g2p")
                nc.tensor.matmul(out=g2p[:TILE_N, :], lhsT=stb[:],
                                 rhs=w2_t[:], start=True, stop=True)
                g2sb = flp.tile([128, D], FP16, tag="g2sb")
                nc.vector.tensor_copy(out=g2sb[:TILE_N, :], in_=g2p[:TILE_N, :])
                o0 = (tg % TPQ) * TILE_N
                nc.scalar.dma_start(
                    out=g2_bounces[tg // TPQ][o0 : o0 + TILE_N, :],
                    in_=g2sb[:TILE_N, :],
                )

            # Flush C: h2 = relu(dinv*acc); logits -> SBUF; max; Exp inline
            # with accum_out into a per-batch [128, TB] sum tile.  A SINGLE
            # Ln per batch then consumes all TB sums (the Tile scheduler
            # would otherwise interleave per-tile Exp/Ln and thrash the
            # activation-table loads, ~1.3us each).
            c_pend = []
            c_state = {"bsum": None}

            def flush_c(tg, acc):
                dv = dinv_t[:TILE_N, tg : tg + 1]
                h2 = flp.tile([128, D], FP16, tag="h1")
                nc.scalar.activation(out=h2[:TILE_N, :], in_=acc[:TILE_N, :],
                                     func=mybir.ActivationFunctionType.Relu,
                                     scale=dv)
                htp = tpsp.tile([128, TILE_N], FP16, tag="stp")
                nc.tensor.transpose(out=htp[:], in_=h2[:TILE_N, :],
                                    identity=ident_t[:TILE_N, :TILE_N])
                htb = flp.tile([128, TILE_N], FP16, tag="stb")
                nc.vector.tensor_copy(out=htb[:], in_=htp[:])
                lg = gpsp.tile([128, DOUT], FP32, tag="g2p")
                nc.tensor.matmul(out=lg[:TILE_N, :], lhsT=htb[:],
                                 rhs=wfc_t[:], start=True, stop=not has_bias)
                if has_bias:
                    nc.tensor.matmul(out=lg[:TILE_N, :],
                                     lhsT=brow_ts[3][:, :TILE_N],
                                     rhs=brow_ts[2][:, :DOUT],
                                     start=False, stop=True)
                lgs = lgp.tile([128, DOUT], FP32, tag="lgs")
                nc.vector.tensor_copy(out=lgs[:TILE_N, :], in_=lg[:TILE_N, :])
                mx = flp.tile([128, 1], FP32, tag="mx")
                nc.vector.tensor_reduce(out=mx[:TILE_N, :], in_=lg[:TILE_N, :],
                                        axis=mybir.AxisListType.X,
                                        op=mybir.AluOpType.max)
                negm = nmp.tile([128, 1], FP32, tag="negm")
                nc.vector.tensor_scalar_mul(out=negm[:TILE_N, :],
                                            in0=mx[:TILE_N, :], scalar1=-1.0)
                if c_state["bsum"] is None:
                    bsum_t = nmp.tile([128, TB], FP32, tag="bsum")
                    c_state["bsum"] = bsum_t
                slot = len(c_pend)
                esc = flp.tile([128, DOUT], FP16, tag="esc")
                nc.scalar.activation(out=esc[:TILE_N, :],
                                     in_=lgs[:TILE_N, :],
                                     func=mybir.ActivationFunctionType.Exp,
                                     bias=negm[:TILE_N, :],
                                     accum_out=c_state["bsum"][:TILE_N,
                                                              slot : slot + 1])
                c_pend.append((tg, lgs, negm, slot))

            def post_batch_c():
                bsum = c_state["bsum"]
                nb = len(c_pend)
                lns = flp.tile([128, TB], FP32, tag="lns")
                nc.scalar.activation(out=lns[:TILE_N, :nb],
                                     in_=bsum[:TILE_N, :nb],
                                     func=mybir.ActivationFunctionType.Ln)
                for tg, lgs, negm, slot in c_pend:
                    nmls = flp.tile([128, 1], FP32, tag="nmls")
                    nc.vector.tensor_tensor(out=nmls[:TILE_N, :],
                                            in0=negm[:TILE_N, :],
                                            in1=lns[:TILE_N, slot : slot + 1],
                                            op=mybir.AluOpType.subtract)
                    ot = flp.tile([128, DOUT], FP32, tag="ot")
                    nc.vector.tensor_tensor(out=ot[:TILE_N, :],
                                            in0=lgs[:TILE_N, :],
                                            in1=nmls[:TILE_N, :]
                                                .to_broadcast([TILE_N, DOUT]),
                                            op=mybir.AluOpType.add)
                    nc.sync.dma_start(
                        out=out[tg * TILE_N : (tg + 1) * TILE_N, :],
                        in_=ot[:TILE_N, :],
                    )
                c_pend.clear()
                c_state["bsum"] = None

            ag2_done = set()

            def ag2_hook(bi):
                tiles_done = (bi + 1) * TB
                for q in range(N_BANKS):
                    if q not in ag2_done and tiles_done >= (q + 1) * TPQ:
                        emit_ag(g2_bounces[q], g2_tables[q])
                        ag2_done.add(q)

            agg_layer(g1_tables, g1_bounces, 0, flush_b, ag_hook=ag2_hook)
            agg_layer(g2_tables, g2_bounces, 1, flush_c, post_batch_c)

    nc.compile()
    return nc


# ---------------------------------------------------------------------------
# Public entry point
# ---------------------------------------------------------------------------

_CACHE = {}


def kernel(x, edge_index, W1, b1, W2, b2, Wfc, bfc):
    x = np.asarray(x, np.float32)
    per_core, meta, dinv, sqdeg = _preprocess(edge_index)

    has_bias = bool(
        np.any(np.asarray(b1)) or np.any(np.asarray(b2)) or np.any(np.asarray(bfc))
    )
    mkey = hashlib.sha1(
        repr((meta["total_chunks"], meta["s_total"], meta["pieces"],
              sorted(meta["tile_runs"].items()), has_bias)).encode()
    ).hexdigest()
    if mkey not in _CACHE:
        _CACHE[mkey] = _build(meta, has_bias)
    nc = _CACHE[mkey]

    xs = (dinv[:, None] * x).T           # [D, n]
    xsT_h = np.ascontiguousarray(xs).astype(np.float16)
    # iota: KM repeats of [0..124, pad] (+PAR_OFF for odd tiles); the
    # pad col value 999 matches neither parity's seg range nor SENT
    row = np.concatenate([np.arange(TILE_N, dtype=np.float32), [999.0]])
    base = np.tile(row, KM)
    iota = np.concatenate([base, base + PAR_OFF]).astype(np.float16)
    iota = np.tile(iota, (128, 1))
    if has_bias:
        brows_np = np.zeros((4, D), np.float32)
        brows_np[0, :] = np.asarray(b1, np.float32)
        brows_np[1, :] = np.asarray(b2, np.float32)
        brows_np[2, : DOUT] = np.asarray(bfc, np.float32)
        brows_np[3, :] = 1.0
        brows_np = brows_np.astype(np.float16)

    in_maps = []
    for c in range(N_CORES):
        s0 = c * SLICE_N
        idx_mat, seg_mat = per_core[c]
        im = {
            "xsT": np.ascontiguousarray(xsT_h[:, s0 : s0 + SLICE_N]),
            "w1": np.asarray(W1, np.float32).astype(np.float16),
            "w2": np.asarray(W2, np.float32).astype(np.float16),
            "wfc": np.asarray(Wfc, np.float32).astype(np.float16),
            "iota": iota,
            "dinvp": _pack_dinv(dinv, s0, np.float32),
            "dinv2p": _pack_dinv(dinv * dinv, s0, np.float32),
            "idx1": _pack_idx(idx_mat, meta),
            "seg1": np.ascontiguousarray(np.repeat(seg_mat.T, 2, axis=1)),
        }
        if has_bias:
            im["brows"] = brows_np
            im["sqdegp"] = sqdeg[s0 : s0 + SLICE_N][None, :].astype(np.float16)
        in_maps.append(im)

    global _last_in_maps
    _last_in_maps = in_maps
    last_exc = None
    for _attempt in range(3):
        try:
            res = bass_utils.run_bass_kernel_spmd(
                nc, in_maps, core_ids=list(range(N_CORES))
            )
            return np.concatenate(
                [res.results[c]["out"] for c in range(N_CORES)], axis=0
            )
        except Exception as e:  # transient device/tunnel errors: retry
            last_exc = e
    raise last_exc


_last_in_maps = None



# revision 17
# speedup vs baseline: 1.0446x; 1.0062x over previous
"""Self-contained Trainium2 Bass kernel for a 2-layer GCN + FC + log_softmax.

Distribution: nodes sharded across 8 NeuronCores (12500 rows each); edges
partitioned by destination node so each core's scatter-add is local; the
per-layer "gather tables" g = D^-1/2 * H * W are exchanged with an on-chip
AllGather; small weights replicated.

Device algorithm per core:
  Stage A : g1 slice = (dinv*x) @ W1 (rows of this core), fp16 -> AllGather
  Agg     : per 125-node dst tile: PSUM += onehot(seg).T @ g1[src]
            (dma_gather of fp16 rows from 4 HBM banks on 4 SWDGE queues,
            one-hot built on VectorE from preloaded seg values, segment-sum
            as TensorE matmul).  Self-loops are NOT gathered: they are a
            diag(dinv) matmul against the core-local g rows.
  Flush B : h1 = relu(dinv*acc); g2 = (dinv*h1) @ W2 -> AllGather
  Flush C : h2 = relu(dinv*acc); logits = h2 @ Wfc; fused log_softmax.
"""
import hashlib
import math

import numpy as np
import ml_dtypes

import concourse.bass as bass
import concourse.mybir as mybir
import concourse.tile as tile
from concourse import bacc, bass_utils

FP16_NP = ml_dtypes.float16 if hasattr(ml_dtypes, "float16") else np.float16

# Problem contract (hardcoded; must match setup_inputs()).
N_NODES = 100000
N_EDGES = 1600000
D = 128
DOUT = 40

N_CORES = 8
TILE_N = 125            # dst nodes per PSUM tile
TB = 10                 # dst tiles per batch
BANK = 25000            # gather table bank rows (int16 index limit 32767)
HALF = 6250             # node rows per core per table half (AG split)
CHUNK = 128             # edges per matmul chunk
MAXC = 12               # chunks per dma_gather call
KM = 8                  # max chunks per one-hot build run
AHEAD = 2               # gather issue lookahead (batches)
PAR_OFF = 500.0         # seg offset for odd tiles (collision guard)
SENT = 1250.0           # seg sentinel (matches no iota value)
OHW = 126               # one-hot row width (125 cols + 1 pad col)

FP32 = mybir.dt.float32
FP16 = mybir.dt.float16
I16 = mybir.dt.int16

SLICE_N = N_NODES // N_CORES            # 12500
QROWS = SLICE_N // 4                    # local rows per AG quarter
N_TILES = SLICE_N // TILE_N             # 100
N_BATCH = N_TILES // TB                 # 10
N_BANKS = (N_NODES + BANK - 1) // BANK  # 4
TPQ = 25                                # dst tiles per bounce quarter


# ---------------------------------------------------------------------------
# Host preprocessing
# ---------------------------------------------------------------------------

def _preprocess(edge_index):
    """Sort edges by (core, batch, bank, tile); build the static chunk
    structure shared by both conv layers plus per-core idx/seg arrays."""
    ei = np.asarray(edge_index, np.int64)
    src, dst = ei[0], ei[1]
    deg = (np.bincount(dst, minlength=N_NODES) + 1.0).astype(np.float32)
    dinv = 1.0 / np.sqrt(deg)
    sqdeg = np.sqrt(deg)

    tile_id = dst // TILE_N                  # global tile 0..799
    core_id = tile_id // N_TILES
    t_loc = tile_id % N_TILES                # tile within core 0..99
    batch_g = t_loc // TB
    # bank q = quarter q of every core's node slice, matching the
    # quarter-AllGather output layout: bank row = core*QROWS + local row
    bank_id = (src % SLICE_N) // QROWS
    idx_local = (src // SLICE_N) * QROWS + (src % SLICE_N) % QROWS
    order = np.lexsort((tile_id, bank_id, batch_g, core_id))
    src_s = src[order]
    dst_s = dst[order]
    tloc_s = t_loc[order]

    # per (core, batch, bank, tile-in-batch) counts
    tl_in_b = tloc_s % TB
    key = (((core_id[order] * N_BATCH + batch_g[order]) * N_BANKS
            + bank_id[order]) * TB + tl_in_b)
    cnt4 = np.bincount(key, minlength=N_CORES * N_BATCH * N_BANKS * TB)
    cnt4 = cnt4.reshape(N_CORES, N_BATCH, N_BANKS, TB)
    cnt3 = cnt4.sum(axis=3)                                # [C, B, K]
    nbk = (-(-cnt3 // CHUNK)).max(axis=0)                  # [B, K] chunks

    # chunk columns + per-tile chunk ranges (union over cores)
    col0 = np.zeros((N_BATCH, N_BANKS), np.int64)
    c = 0
    for b in range(N_BATCH):
        for k in range(N_BANKS):
            col0[b, k] = c
            c += int(nbk[b, k])
    total_chunks = c

    ends = np.cumsum(cnt4, axis=3)                         # [C,B,K,TB]
    starts = ends - cnt4
    lo = np.where(cnt4 > 0, starts // CHUNK, np.iinfo(np.int64).max)
    hi = np.where(cnt4 > 0, -(-ends // CHUNK), 0)
    lo = lo.min(axis=0)                                    # [B,K,TB]
    hi = hi.max(axis=0)

    # runs per (batch, tile): contiguous chunk ranges split to <= KM
    tile_runs = {}
    for b in range(N_BATCH):
        for tl in range(TB):
            runs = []
            for k in range(N_BANKS):
                l, h = int(lo[b, k, tl]), int(hi[b, k, tl])
                if h <= l:
                    continue
                base = int(col0[b, k])
                x = l
                while x < h:
                    r = min(KM, h - x)
                    runs.append((base + x, r))
                    x += r
            tile_runs[(b, tl)] = runs

    # gather call pieces: split each (batch, bank) into two equal halves so
    # the round-robin SWDGE queues see balanced generation work
    pieces = []                      # (b, k, row0, rows, gl, col_start, off16)
    batch_pieces = [[] for _ in range(N_BATCH)]
    s16 = 0
    for b in range(N_BATCH):
        tmp = []                     # (si, k, piece_idx) for interleave sort
        for k in range(N_BANKS):
            nb = int(nbk[b, k])
            rows = BANK
            nsplit = max(1, -(-nb // MAXC))
            g0 = 0
            for si in range(nsplit):
                gl = (nb - g0) // (nsplit - si)
                if gl == 0:
                    continue
                tmp.append((si, k, len(pieces)))
                pieces.append((b, k, 0, rows, gl,
                               int(col0[b, k]) + g0, s16))
                s16 += gl * 8
                g0 += gl
        # issue order (si, k): first slice of every bank before any second
        # slice, so all 4 SWDGE queues start draining immediately
        batch_pieces[b] = [pi for _, _, pi in sorted(tmp)]
    s_total = s16

    # per-core idx/seg arrays in chunk-column order
    per_core = []
    seg_all = ((dst_s % TILE_N) + PAR_OFF * (tloc_s % 2)).astype(np.float16)
    idx_all = idx_local[order].astype(np.int16)
    # group start offset per (core, batch, bank) in sorted edge order
    grp_sizes = cnt3.reshape(-1)
    grp_off = np.concatenate([[0], np.cumsum(grp_sizes)])
    for cidx in range(N_CORES):
        idx_mat = np.zeros((total_chunks, CHUNK), np.int16)
        seg_mat = np.full((total_chunks, CHUNK), SENT, np.float16)
        for b in range(N_BATCH):
            for k in range(N_BANKS):
                g = (cidx * N_BATCH + b) * N_BANKS + k
                o0, m = int(grp_off[g]), int(grp_sizes[g])
                nb = int(nbk[b, k])
                ii = np.zeros(nb * CHUNK, np.int16)
                ss = np.full(nb * CHUNK, SENT, np.float16)
                ii[:m] = idx_all[o0 : o0 + m]
                ss[:m] = seg_all[o0 : o0 + m]
                c0 = int(col0[b, k])
                idx_mat[c0 : c0 + nb] = ii.reshape(nb, CHUNK)
                seg_mat[c0 : c0 + nb] = ss.reshape(nb, CHUNK)
        per_core.append((idx_mat, seg_mat))

    meta = {
        "total_chunks": total_chunks, "s_total": s_total,
        "pieces": pieces, "batch_pieces": batch_pieces,
        "tile_runs": tile_runs,
    }
    return per_core, meta, dinv, sqdeg


def _pack_idx(idx_mat, meta):
    """Wrap chunk-major indices into the dma_gather [16, n/16] layout per
    (batch, bank) block, concatenated, replicated to 128 partitions."""
    blocks = []
    for b, k, row0, rows, gl, cs, off16 in meta["pieces"]:
        flat = idx_mat[cs : cs + gl].reshape(-1)           # [gl*128]
        blocks.append(flat.reshape(-1, 16).T)              # [16, gl*8]
    packed = np.concatenate(blocks, axis=1)
    assert packed.shape[1] == meta["s_total"]
    return np.tile(packed, (8, 1)).copy()                  # [128, S]


def _pack_dinv(v, slice0, dtype):
    """[128, n_tiles]: partition p, col t = v[slice0 + t*TILE_N + p]."""
    out = np.zeros((128, N_TILES), dtype)
    sl = v[slice0 : slice0 + N_TILES * TILE_N].reshape(N_TILES, TILE_N)
    out[:TILE_N, :] = sl.T
    return out


# ---------------------------------------------------------------------------
# Device kernel builder
# ---------------------------------------------------------------------------

def _build(meta, has_bias):
    total_chunks = meta["total_chunks"]
    s_total = meta["s_total"]
    pieces = meta["pieces"]
    batch_pieces = meta["batch_pieces"]
    tile_runs = meta["tile_runs"]
    n_a_tiles = math.ceil(SLICE_N / 128)

    nc = bacc.Bacc("TRN2", target_bir_lowering=False, debug=False,
                   num_devices=N_CORES, num_swdge_queues=4,
                   dynamic_dma_scratch_size=16384)

    # inputs
    xsT = nc.dram_tensor("xsT", [D, SLICE_N], FP16, kind="ExternalInput")
    w1 = nc.dram_tensor("w1", [D, D], FP16, kind="ExternalInput")
    w2 = nc.dram_tensor("w2", [D, D], FP16, kind="ExternalInput")
    wfc = nc.dram_tensor("wfc", [D, DOUT], FP16, kind="ExternalInput")
    iota_in = nc.dram_tensor("iota", [128, 2 * KM * OHW], FP16,
                             kind="ExternalInput")
    dinvp = nc.dram_tensor("dinvp", [128, N_TILES], FP32, kind="ExternalInput")
    dinv2p = nc.dram_tensor("dinv2p", [128, N_TILES], FP32,
                            kind="ExternalInput")
    idx1 = nc.dram_tensor("idx1", [128, s_total], I16, kind="ExternalInput")
    seg1 = nc.dram_tensor("seg1", [128, 2 * total_chunks], FP16,
                          kind="ExternalInput")
    brows = (nc.dram_tensor("brows", [4, D], FP16, kind="ExternalInput")
             if has_bias else None)
    sqdegp = (nc.dram_tensor("sqdegp", [1, SLICE_N], FP16,
                             kind="ExternalInput") if has_bias else None)

    out = nc.dram_tensor("out", [SLICE_N, DOUT], FP32, kind="ExternalOutput")

    # internal dram: per-quarter bounce + gather-table tensors so each
    # quarter AllGather and its consumers have clean tensor-level deps
    g1_bounces = [nc.dram_tensor(f"g1_bounce{q}", [QROWS, D], FP16)
                  for q in range(N_BANKS)]
    g2_bounces = [nc.dram_tensor(f"g2_bounce{q}", [QROWS, D], FP16)
                  for q in range(N_BANKS)]
    g1_tables = [nc.dram_tensor(f"g1_table{q}", [BANK, D], FP16,
                                addr_space="Shared") for q in range(N_BANKS)]
    g2_tables = [nc.dram_tensor(f"g2_table{q}", [BANK, D], FP16,
                                addr_space="Shared") for q in range(N_BANKS)]

    warm_in = nc.dram_tensor("warm_in", [1, 16], FP16)
    warm_out = nc.dram_tensor("warm_out", [N_CORES, 16], FP16,
                              addr_space="Shared")

    with tile.TileContext(nc) as tc:
        with (
            tc.tile_pool(name="const", bufs=1) as constp,
            tc.tile_pool(name="aio", bufs=4) as aio,
            tc.tile_pool(name="msg", bufs=48) as msgp,
            tc.tile_pool(name="mp", bufs=6) as mp,
            tc.tile_pool(name="gs", bufs=4) as gsp,
            tc.tile_pool(name="fl", bufs=4) as flp,
            tc.tile_pool(name="lg", bufs=12) as lgp,
            tc.tile_pool(name="nm", bufs=12) as nmp,
            tc.tile_pool(name="acc", bufs=4, space="PSUM") as accp,
            tc.tile_pool(name="tps", bufs=2, space="PSUM") as tpsp,
            tc.tile_pool(name="gps", bufs=2, space="PSUM") as gpsp,
        ):
            # a minimal first collective: the runtime attaches its ~40-50us
            # init BARRIER to the first cc op, so issue one with no deps and
            # let it overlap the constant loads + stage A compute
            nc.gpsimd.collective_compute(
                "AllGather", mybir.AluOpType.bypass,
                ins=[warm_in[:, :]], outs=[warm_out[:, :]],
                replica_groups=[list(range(N_CORES))],
            )
            # constants
            w1_t = constp.tile([D, D], FP16, tag="w1")
            nc.sync.dma_start(out=w1_t[:], in_=w1[:, :])
            w2_t = constp.tile([D, D], FP16, tag="w2")
            nc.sync.dma_start(out=w2_t[:], in_=w2[:, :])
            wfc_t = constp.tile([D, DOUT], FP16, tag="wfc")
            nc.sync.dma_start(out=wfc_t[:], in_=wfc[:, :])
            iota_t = constp.tile([128, 2 * KM * OHW], FP16, tag="iota")
            nc.sync.dma_start(out=iota_t[:], in_=iota_in[:, :])
            dinv_t = constp.tile([128, N_TILES], FP32, tag="dinvp")
            nc.sync.dma_start(out=dinv_t[:], in_=dinvp[:, :])
            dinv2_t = constp.tile([128, N_TILES], FP32, tag="dinv2p")
            nc.sync.dma_start(out=dinv2_t[:], in_=dinv2p[:, :])
            idx_t = constp.tile([128, s_total], I16, tag="idx")
            nc.sync.dma_start(out=idx_t[:], in_=idx1[:, :])
            seg_t = constp.tile([128, 2 * total_chunks], FP16, tag="seg")
            nc.sync.dma_start(out=seg_t[:], in_=seg1[:, :])
            if has_bias:
                brow_ts = []
                for r in range(4):
                    bt = constp.tile([1, D], FP16, tag=f"brow{r}")
                    nc.sync.dma_start(out=bt[:], in_=brows[r : r + 1, :])
                    brow_ts.append(bt)
                sqdeg_t = constp.tile([1, SLICE_N], FP16, tag="sqdegp")
                nc.sync.dma_start(out=sqdeg_t[:], in_=sqdegp[:, :])
            ident_t = constp.tile([128, 128], FP16, tag="ident")
            from concourse.masks import make_identity
            make_identity(nc, ident_t[:])

            def emit_ag(bounce, table):
                nc.gpsimd.collective_compute(
                    "AllGather", mybir.AluOpType.bypass,
                    ins=[bounce[:, :]], outs=[table[:, :]],
                    replica_groups=[list(range(N_CORES))],
                )

            # ---------------- Stage A ----------------
            # per AG quarter: compute g1 rows, then AllGather that quarter
            # immediately so layer-1 gathers on bank q start ~3 quarters
            # earlier than a monolithic AllGather would allow
            BL = 4
            for q in range(N_BANKS):
                st = 0
                while st < QROWS:
                    bw = min(128 * BL, QROWS - st)
                    nch = math.ceil(bw / 128)
                    g0 = q * QROWS + st
                    xt = aio.tile([D, 128 * BL], FP16, tag="xt")
                    nc.sync.dma_start(out=xt[:, :bw], in_=xsT[:, g0 : g0 + bw])
                    gsb = aio.tile([128, BL, D], FP16, tag="gsb")
                    for i in range(nch):
                        w = min(128, bw - i * 128)
                        ps = accp.tile([128, D], FP32, tag="acc")
                        nc.tensor.matmul(out=ps[:w, :],
                                         lhsT=xt[:, i * 128 : i * 128 + w],
                                         rhs=w1_t[:], start=True, stop=True)
                        nc.vector.tensor_copy(out=gsb[:w, i, :], in_=ps[:w, :])
                    full = (bw // 128) * 128
                    if full:
                        nc.sync.dma_start(
                            out=g1_bounces[q][st : st + full, :]
                                .rearrange("(b p) d -> p b d", p=128),
                            in_=gsb[:, : full // 128, :])
                    if bw > full:
                        w = bw - full
                        nc.sync.dma_start(
                            out=g1_bounces[q][st + full : st + bw, :],
                            in_=gsb[:w, full // 128, :])
                    st += bw
                emit_ag(g1_bounces[q], g1_tables[q])

            # ---------------- aggregation layers ----------------
            qrr = [0]
            _regs = {}

            def nidx_reg(v):
                if v not in _regs:
                    _regs[v] = nc.gpsimd.to_reg(v)
                return _regs[v]

            def issue_piece(tables, pi, cmap):
                b, k, row0, rows, gl, cs, off16 = pieces[pi]
                mt = msgp.tile([128, gl, D], FP16, tag="msg")
                # queue = bank so a not-yet-AllGathered bank only stalls its
                # own SWDGE queue
                nc.gpsimd.dma_gather(
                    out_ap=mt[:],
                    in_ap=tables[k][row0 : row0 + rows, :],
                    idxs_ap=idx_t[:, off16 : off16 + gl * 8],
                    num_idxs=gl * CHUNK,
                    num_idxs_reg=nidx_reg(gl * CHUNK),
                    elem_size=D,
                    single_packet=False,
                    queue_num=k,
                )
                for j in range(gl):
                    cmap[cs + j] = (mt, j)

            def issue_batch(tables, bi):
                cmap = {}
                for pi in batch_pieces[bi]:
                    issue_piece(tables, pi, cmap)
                return cmap

            def process_batch(bi, cmap, bounces, brow_idx, flush):
                for tl in range(TB):
                    tg = bi * TB + tl
                    t0 = (tg % TPQ) * TILE_N
                    acc = accp.tile([128, D], FP32, tag="acc")
                    # self-loop: identity @ g_local (flush's *dinv[d] covers
                    # the dinv[d]^2 * h[d] = dinv[d] * g[d] self message)
                    gself = gsp.tile([128, D], FP16, tag="gself")
                    nc.sync.dma_start(out=gself[:TILE_N, :],
                                      in_=bounces[tg // TPQ]
                                          [t0 : t0 + TILE_N, :])
                    nc.tensor.matmul(out=acc[:TILE_N, :],
                                     lhsT=ident_t[:TILE_N, :TILE_N],
                                     rhs=gself[:TILE_N, :],
                                     start=True, stop=False)
                    if has_bias:
                        nc.tensor.matmul(
                            out=acc[:TILE_N, :],
                            lhsT=sqdeg_t[:, tg * TILE_N : tg * TILE_N + TILE_N],
                            rhs=brow_ts[brow_idx][:, :],
                            start=False, stop=False,
                        )
                    runs = tile_runs[(bi, tl)]
                    nch = sum(r for _, r in runs)
                    par = tl % 2
                    io0 = par * KM * OHW
                    ci = 0
                    for col0, r in runs:
                        # one-hot [128, r, 126] via one tensor_tensor
                        # per run.  seg is stored duplicated in PAIRS so the
                        # innermost AP dim of every operand is packed stride-1
                        # fp16 -> DVE 2x_1p mode (a plain seg broadcast has
                        # inner stride 0 and falls back to 1x).  Col 125 of
                        # each one-hot row compares against a 999 pad value
                        # (never matches); the matmul uses cols 0..124.
                        mtile = mp.tile([128, KM, OHW], FP16, tag="m")
                        nc.vector.tensor_tensor(
                            out=mtile[:, :r, :]
                                .rearrange("p r (x2 xi) -> p r x2 xi", xi=2),
                            in0=seg_t[:, 2 * col0 : 2 * (col0 + r)]
                                .rearrange("p (r o xi) -> p r o xi",
                                           o=1, xi=2)
                                .to_broadcast([128, r, OHW // 2, 2]),
                            in1=iota_t[:, io0 : io0 + r * OHW]
                                .rearrange("p (r x2 xi) -> p r x2 xi",
                                           r=r, xi=2),
                            op=mybir.AluOpType.is_equal,
                        )
                        for i in range(r):
                            mt, j = cmap[col0 + i]
                            ci += 1
                            nc.tensor.matmul(
                                out=acc[:TILE_N, :],
                                lhsT=mtile[:, i, :TILE_N],
                                rhs=mt[:, j, :],
                                start=False,
                                stop=(ci == nch),
                            )
                    flush(tg, acc)

            def agg_layer(tables, bounces, brow_idx, flush,
                          post_batch=None, ag_hook=None):
                issued = {}
                for bi in range(min(AHEAD + 1, N_BATCH)):
                    issued[bi] = issue_batch(tables, bi)
                for bi in range(N_BATCH):
                    process_batch(bi, issued.pop(bi), bounces, brow_idx, flush)
                    if post_batch is not None:
                        post_batch()
                    if ag_hook is not None:
                        ag_hook(bi)
                    nxt = bi + AHEAD + 1
                    if nxt < N_BATCH:
                        issued[nxt] = issue_batch(tables, nxt)

            # Flush B: s = dinv*relu(dinv*acc) = relu(dinv^2*acc);
            # g2 = s @ W2
            def flush_b(tg, acc):
                dv2 = dinv2_t[:TILE_N, tg : tg + 1]
                s = flp.tile([128, D], FP16, tag="s")
                nc.scalar.activation(out=s[:TILE_N, :], in_=acc[:TILE_N, :],
                                     func=mybir.ActivationFunctionType.Relu,
                                     scale=dv2)
                stp = tpsp.tile([128, TILE_N], FP16, tag="stp")
                nc.tensor.transpose(out=stp[:], in_=s[:TILE_N, :],
                                    identity=ident_t[:TILE_N, :TILE_N])
                stb = flp.tile([128, TILE_N], FP16, tag="stb")
                nc.vector.tensor_copy(out=stb[:], in_=stp[:])
                g2p = gpsp.tile([128, D], FP32, tag="g2p")
                nc.tensor.matmul(out=g2p[:TILE_N, :], lhsT=stb[:],
                                 rhs=w2_t[:], start=True, stop=True)
                g2sb = flp.tile([128, D], FP16, tag="g2sb")
                nc.vector.tensor_copy(out=g2sb[:TILE_N, :], in_=g2p[:TILE_N, :])
                o0 = (tg % TPQ) * TILE_N
                nc.scalar.dma_start(
                    out=g2_bounces[tg // TPQ][o0 : o0 + TILE_N, :],
                    in_=g2sb[:TILE_N, :],
                )

            # Flush C: h2 = relu(dinv*acc); logits -> SBUF; max; Exp inline
            # with accum_out into a per-batch [128, TB] sum tile.  A SINGLE
            # Ln per batch then consumes all TB sums (the Tile scheduler
            # would otherwise interleave per-tile Exp/Ln and thrash the
            # activation-table loads, ~1.3us each).
            c_pend = []
            c_state = {"bsum": None}

            def flush_c(tg, acc):
                dv = dinv_t[:TILE_N, tg : tg + 1]
                h2 = flp.tile([128, D], FP16, tag="h1")
                nc.scalar.activation(out=h2[:TILE_N, :], in_=acc[:TILE_N, :],
                                     func=mybir.ActivationFunctionType.Relu,
                                     scale=dv)
                htp = tpsp.tile([128, TILE_N], FP16, tag="stp")
                nc.tensor.transpose(out=htp[:], in_=h2[:TILE_N, :],
                                    identity=ident_t[:TILE_N, :TILE_N])
                htb = flp.tile([128, TILE_N], FP16, tag="stb")
                nc.vector.tensor_copy(out=htb[:], in_=htp[:])
                lg = gpsp.tile([128, DOUT], FP32, tag="g2p")
                nc.tensor.matmul(out=lg[:TILE_N, :], lhsT=htb[:],
                                 rhs=wfc_t[:], start=True, stop=not has_bias)
                if has_bias:
                    nc.tensor.matmul(out=lg[:TILE_N, :],
                                     lhsT=brow_ts[3][:, :TILE_N],
                                     rhs=brow_ts[2][:, :DOUT],
                                     start=False, stop=True)
                lgs = lgp.tile([128, DOUT], FP32, tag="lgs")
                nc.vector.tensor_copy(out=lgs[:TILE_N, :], in_=lg[:TILE_N, :])
                mx = flp.tile([128, 1], FP32, tag="mx")
                nc.vector.tensor_reduce(out=mx[:TILE_N, :], in_=lg[:TILE_N, :],
                                        axis=mybir.AxisListType.X,
                                        op=mybir.AluOpType.max)
                negm = nmp.tile([128, 1], FP32, tag="negm")
                nc.vector.tensor_scalar_mul(out=negm[:TILE_N, :],
                                            in0=mx[:TILE_N, :], scalar1=-1.0)
                if c_state["bsum"] is None:
                    bsum_t = nmp.tile([128, TB], FP32, tag="bsum")
                    c_state["bsum"] = bsum_t
                slot = len(c_pend)
                esc = flp.tile([128, DOUT], FP16, tag="esc")
                nc.scalar.activation(out=esc[:TILE_N, :],
                                     in_=lgs[:TILE_N, :],
                                     func=mybir.ActivationFunctionType.Exp,
                                     bias=negm[:TILE_N, :],
                                     accum_out=c_state["bsum"][:TILE_N,
                                                              slot : slot + 1])
                c_pend.append((tg, lgs, negm, slot))

            def post_batch_c():
                bsum = c_state["bsum"]
                nb = len(c_pend)
                lns = flp.tile([128, TB], FP32, tag="lns")
                nc.scalar.activation(out=lns[:TILE_N, :nb],
                                     in_=bsum[:TILE_N, :nb],
                                     func=mybir.ActivationFunctionType.Ln)
                for tg, lgs, negm, slot in c_pend:
                    nmls = flp.tile([128, 1], FP32, tag="nmls")
                    nc.vector.tensor_tensor(out=nmls[:TILE_N, :],
                                            in0=negm[:TILE_N, :],
                                            in1=lns[:TILE_N, slot : slot + 1],
                                            op=mybir.AluOpType.subtract)
                    ot = flp.tile([128, DOUT], FP32, tag="ot")
                    nc.vector.tensor_tensor(out=ot[:TILE_N, :],
                                            in0=lgs[:TILE_N, :],
                                            in1=nmls[:TILE_N, :]
                                                .to_broadcast([TILE_N, DOUT]),
                                            op=mybir.AluOpType.add)
                    nc.sync.dma_start(
                        out=out[tg * TILE_N : (tg + 1) * TILE_N, :],
                        in_=ot[:TILE_N, :],
                    )
                c_pend.clear()
                c_state["bsum"] = None

            ag2_done = set()

            def ag2_hook(bi):
                tiles_done = (bi + 1) * TB
                for q in range(N_BANKS):
                    if q not in ag2_done and tiles_done >= (q + 1) * TPQ:
                        emit_ag(g2_bounces[q], g2_tables[q])
                        ag2_done.add(q)

            agg_layer(g1_tables, g1_bounces, 0, flush_b, ag_hook=ag2_hook)
            agg_layer(g2_tables, g2_bounces, 1, flush_c, post_batch_c)

    nc.compile()
    return nc


# ---------------------------------------------------------------------------
# Public entry point
# ---------------------------------------------------------------------------

_CACHE = {}


def kernel(x, edge_index, W1, b1, W2, b2, Wfc, bfc):
    x = np.asarray(x, np.float32)
    per_core, meta, dinv, sqdeg = _preprocess(edge_index)

    has_bias = bool(
        np.any(np.asarray(b1)) or np.any(np.asarray(b2)) or np.any(np.asarray(bfc))
    )
    mkey = hashlib.sha1(
        repr((meta["total_chunks"], meta["s_total"], meta["pieces"],
              sorted(meta["tile_runs"].items()), has_bias)).encode()
    ).hexdigest()
    if mkey not in _CACHE:
        _CACHE[mkey] = _build(meta, has_bias)
    nc = _CACHE[mkey]

    xs = (dinv[:, None] * x).T           # [D, n]
    xsT_h = np.ascontiguousarray(xs).astype(np.float16)
    # iota: KM repeats of [0..124, pad] (+PAR_OFF for odd tiles); the
    # pad col value 999 matches neither parity's seg range nor SENT
    row = np.concatenate([np.arange(TILE_N, dtype=np.float32), [999.0]])
    base = np.tile(row, KM)
    iota = np.concatenate([base, base + PAR_OFF]).astype(np.float16)
    iota = np.tile(iota, (128, 1))
    if has_bias:
        brows_np = np.zeros((4, D), np.float32)
        brows_np[0, :] = np.asarray(b1, np.float32)
        brows_np[1, :] = np.asarray(b2, np.float32)
        brows_np[2, : DOUT] = np.asarray(bfc, np.float32)
        brows_np[3, :] = 1.0
        brows_np = brows_np.astype(np.float16)

    in_maps = []
    for c in range(N_CORES):
        s0 = c * SLICE_N
        idx_mat, seg_mat = per_core[c]
        im = {
            "xsT": np.ascontiguousarray(xsT_h[:, s0 : s0 + SLICE_N]),
            "w1": np.asarray(W1, np.float32).astype(np.float16),
            "w2": np.asarray(W2, np.float32).astype(np.float16),
            "wfc": np.asarray(Wfc, np.float32).astype(np.float16),
            "iota": iota,
            "dinvp": _pack_dinv(dinv, s0, np.float32),
            "dinv2p": _pack_dinv(dinv * dinv, s0, np.float32),
            "idx1": _pack_idx(idx_mat, meta),
            "seg1": np.ascontiguousarray(np.repeat(seg_mat.T, 2, axis=1)),
        }
        if has_bias:
            im["brows"] = brows_np
            im["sqdegp"] = sqdeg[s0 : s0 + SLICE_N][None, :].astype(np.float16)
        in_maps.append(im)

    global _last_in_maps
    _last_in_maps = in_maps
    last_exc = None
    for _attempt in range(3):
        try:
            res = bass_utils.run_bass_kernel_spmd(
                nc, in_maps, core_ids=list(range(N_CORES))
            )
            return np.concatenate(
                [res.results[c]["out"] for c in range(N_CORES)], axis=0
            )
        except Exception as e:  # transient device/tunnel errors: retry
            last_exc = e
    raise last_exc


_last_in_maps = None

